# revision 47
# baseline (speedup 1.0000x reference)
"""GCN block (2-layer) Trainium2 Bass kernel.

Math (per B*T slice, shared graph):
  t2 = relu(A @ (X @ W1) + b1);  out = sigmoid(A @ t2 @ W2 + b2)
  A = D^-1/2 (Adj + I) D^-1/2  (PyG gcn_norm, counts edge multiplicity)

Device mapping (per core, 8-way dst-node sharding, N padded 10000->10240):
  * M = Adj + I is applied as dense fp8 (exact small ints) PE matmuls in
    DoubleRow mode (K=256).  Each core owns 10 of the 80 dst node blocks.
    M^T slabs [128 src, 2, 1280 dst] stay SBUF-resident and serve BOTH
    layers: layer 1 uses a [128,2,128] dst-column slice as the stationary
    operand (out = t2, node-major); layer 2 uses the full slab as the
    MOVING operand with the t2 block as stationary, so the A-output lands
    TRANSPOSED ([cols, dst]) and feeds W2 + sigmoid directly — no DRAM
    round-trip / DMA transpose for the W2 stage.
  * W1 is sharded: each core transforms only its 10 src blocks, then the
    fp8 xw activations are AllGather'd; same for the relu'd t2 between
    the layers.  Work is split into 3 column chunks (512 of 1536 cols =
    4 slice-pairs) so the two AllGathers pipeline under the A-stage
    matmuls of neighbouring chunks.  The 6-buffer quad pool doubles as a
    scheduling throttle: later quad-set loads block on tile reuse and
    land just in time, keeping early DMA bandwidth for the M^T stream.
  * dinv factors: src side folded into X on host; dst side applied at the
    layer-1 drain (per-partition scale) and at the layer-2 drain (row-
    replicated dinv tile, elementwise on the free dim).

Timing build (with_collective=False) replaces each AllGather with local
DMA traffic equivalent to what the real path costs the local DMA engines:
the post-collective SBUF loads of the full gathered activations.
"""
import time

import numpy as np
import ml_dtypes

import concourse.bacc as bacc
import concourse.mybir as mybir
import concourse.tile as tile
from concourse.bass_utils import run_bass_kernel_spmd

N_CORES = 8
N = 10000
NP = 10240            # padded nodes
NB = NP // 128        # 80 node blocks
NB2 = NB // 2         # 40 src-block pairs (DoubleRow K=256)
BPC = NB // N_CORES   # 10 dst blocks per core
B, T, C = 2, 12, 64
S = B * T             # 24 slices
F = S * C             # 1536 free columns, col = pl*128 + h*64 + c
PAIRS = S // 2        # 12 slice pairs (s = 2*pl + h)
NCH = 3               # column chunks
FC = F // NCH         # 512 cols = 4 pairs per chunk
PC = PAIRS // NCH     # 4 pairs per chunk
NDST = BPC * 128      # 1280 dst nodes per core
CHAINS = ((0, 512), (512, 512), (1024, 256))  # dst chains for layer 2
QT = 10               # quad tiles per set ([128, 8, FC] each)
QB = NB // QT         # 16 src blocks per quad tile

f32 = mybir.dt.float32
bf16 = mybir.dt.bfloat16
fp8 = mybir.dt.float8e4
DR = mybir.MatmulPerfMode.DoubleRow
AF = mybir.ActivationFunctionType
ALU = mybir.AluOpType


def build_program(with_collective=True, nc_hook=None):
    nc = bacc.Bacc("TRN2", target_bir_lowering=False, debug=False,
                   num_devices=N_CORES)
    if nc_hook is not None:
        nc_hook(nc)

    # X blocks for this core's 10 src blocks: [b][128=(h,cin)][pl*128+node]
    xb_ext = nc.dram_tensor("XB", [BPC, 128, PAIRS * 128], bf16,
                            kind="ExternalInput")
    # M^T slabs: [j2][p_src][e*1280 + dst], fp8 exact ints
    mt_ext = nc.dram_tensor("MT", [NB2, 128, 2 * NDST], fp8,
                            kind="ExternalInput")
    w1_ext = nc.dram_tensor("W1d", [128, 128], bf16, kind="ExternalInput")
    w2_ext = nc.dram_tensor("W2d", [128, 128], bf16, kind="ExternalInput")
    b1_ext = nc.dram_tensor("B1", [128, FC], bf16, kind="ExternalInput")
    b2_ext = nc.dram_tensor("B2", [128, 1], f32, kind="ExternalInput")
    di_ext = nc.dram_tensor("DI", [128, BPC], f32, kind="ExternalInput")
    dr_ext = nc.dram_tensor("DRW", [128, NDST], bf16, kind="ExternalInput")
    out_ext = nc.dram_tensor("OUT", [PAIRS, 128, NDST], bf16,
                             kind="ExternalOutput")

    with tile.TileContext(nc) as tc:
        with (
            tc.tile_pool(name="consts", bufs=1) as consts,
            tc.tile_pool(name="mt", bufs=NB2) as pool_mt,
            tc.tile_pool(name="xb", bufs=4) as pool_xb,
            tc.tile_pool(name="quads", bufs=9) as pool_q,
            tc.tile_pool(name="stage", bufs=2) as pool_st,
            tc.tile_pool(name="u", bufs=2) as pool_u,
            tc.tile_pool(name="s2", bufs=3) as pool_s2,
            tc.tile_pool(name="ot", bufs=3) as pool_ot,
            tc.tile_pool(name="pa", bufs=6, space="PSUM") as pool_pa,
            tc.tile_pool(name="p2", bufs=2, space="PSUM") as pool_p2,
            tc.tile_pool(name="dram", bufs=1, space="DRAM") as dram,
        ):
            # ---- constants ----
            w1t = consts.tile([128, 128], bf16, tag="w1")
            nc.sync.dma_start(w1t[:], w1_ext[:])
            w2t = consts.tile([128, 128], bf16, tag="w2")
            nc.sync.dma_start(w2t[:], w2_ext[:])
            b1t = consts.tile([128, FC], bf16, tag="b1")
            nc.sync.dma_start(b1t[:], b1_ext[:])
            b2t = consts.tile([128, 1], f32, tag="b2")
            nc.sync.dma_start(b2t[:], b2_ext[:])
            dit = consts.tile([128, BPC], f32, tag="di")
            nc.sync.dma_start(dit[:], di_ext[:])
            # ---- M^T slabs, SBUF-resident, serve both layers ----
            mt = []
            for j2 in range(NB2):
                m = pool_mt.tile([128, 2, NDST], fp8, tag="mt")
                nc.sync.dma_start(m[:].rearrange("p a d -> p (a d)"),
                                  mt_ext[j2])
                mt.append(m)
            drt = consts.tile([128, NDST], bf16, tag="dr")
            with tc.tile_wait_until(0.100):
                nc.sync.dma_start(drt[:], dr_ext[:])

            # ---- DRAM intermediates (per chunk) ----
            # over-allocated to QB*128 rows so the timing build's AllGather
            # stand-in can source a full quad tile in one DMA
            LR = max(QB * 128, NDST)
            xw_loc = [dram.tile([LR, FC], fp8, tag=f"xwl{q}",
                                name=f"xwl{q}") for q in range(NCH)]
            t2_loc = [dram.tile([LR, FC], fp8, tag=f"t2l{q}",
                                name=f"t2l{q}") for q in range(NCH)]
            if with_collective:
                xw_full = [dram.tile([NP, FC], fp8, tag=f"xwf{q}", name=f"xwf{q}",
                                     addr_space="Shared")
                           for q in range(NCH)]
                t2_full = [dram.tile([NP, FC], fp8, tag=f"t2f{q}", name=f"t2f{q}",
                                     addr_space="Shared")
                           for q in range(NCH)]

            def w1_chunk(q, xb_wait=None):
                """xw(q) = (X @ W1) for this core's 10 blocks, cols of q."""
                big = pool_st.tile([128, BPC, FC], fp8, tag="st")
                for h in range(2):
                    xb = pool_xb.tile([128, BPC // 2, FC], bf16, tag="xb")
                    with tc.tile_wait_until(xb_wait or 0,
                                            enable=xb_wait is not None):
                        nc.scalar.dma_start(
                            xb[:],
                            xb_ext[h * 5:h * 5 + 5, :, q * FC:(q + 1) * FC]
                            .rearrange("a p d -> p a d"))
                    for i in range(BPC // 2):
                        b = h * 5 + i
                        ps = pool_p2.tile([128, FC], f32, tag="p2")
                        for pl in range(PC):
                            nc.tensor.matmul(
                                ps[:, pl * 128:(pl + 1) * 128],
                                xb[:, i, pl * 128:(pl + 1) * 128], w1t[:],
                                start=True, stop=True)
                        if b % 2 == 0:
                            nc.vector.tensor_scalar_mul(big[:, b, :], ps[:],
                                                        1.0)
                        else:
                            nc.scalar.activation(big[:, b, :], ps[:], AF.Copy)
                nc.scalar.dma_start(
                    xw_loc[q][:NDST, :].rearrange("(b p) f -> p b f", p=128),
                    big[:])
                if with_collective:
                    nc.gpsimd.collective_compute(
                        "AllGather", ALU.bypass,
                        replica_groups=[list(range(N_CORES))],
                        ins=[xw_loc[q][:NDST, :]], outs=[xw_full[q][:]])

            def quad_load(q, full, loc):
                """Load the gathered [NP, FC] activations into a 5-tile quad
                set.  Timing build: equivalent local-DMA traffic sourced from
                the local shard (content unused for timing)."""
                tiles = []
                for g in range(QT):
                    qt = pool_q.tile([128, QB, FC], fp8, tag="quad")
                    eng = nc.scalar if g % 2 == 0 else nc.sync
                    if with_collective:
                        eng.dma_start(
                            qt[:],
                            full[q][g * QB * 128:(g + 1) * QB * 128, :]
                            .rearrange("(a p) f -> p a f", p=128))
                    else:
                        eng.dma_start(
                            qt[:],
                            loc[q][:QB * 128, :]
                            .rearrange("(a p) f -> p a f", p=128))
                    tiles.append(qt)
                return tiles

            def a_slice(tiles, j2, c0, w):
                g, a = (2 * j2) // QB, (2 * j2) % QB
                return tiles[g][:, a:a + 2, c0:c0 + w]

            def l1a_chunk(q, xwq, mid=None):
                """t2(q) = dinv * relu(dinv * (M @ xw(q)) + b1), node-major."""
                big = pool_st.tile([128, BPC, FC], fp8, tag="st")
                for g0 in (0, 5):
                    if g0 == 5 and mid is not None:
                        mid()
                    pss = [pool_pa.tile([128, FC], f32, tag="pa", name=f"pa{q}_{g0}_{i}")
                           for i in range(5)]
                    for j2 in range(NB2):
                        for i in range(5):
                            bi = g0 + i
                            nc.tensor.matmul(
                                pss[i][:],
                                mt[j2][:, :, bi * 128:(bi + 1) * 128],
                                a_slice(xwq, j2, 0, FC),
                                start=(j2 == 0), stop=(j2 == NB2 - 1),
                                perf_mode=DR)
                    for i in range(5):
                        bi = g0 + i
                        u = pool_u.tile([128, FC], bf16, tag="u")
                        nc.vector.scalar_tensor_tensor(
                            u[:], pss[i][:], dit[:, bi:bi + 1], b1t[:],
                            ALU.mult, ALU.add)
                        nc.scalar.activation(big[:, bi, :], u[:], AF.Relu,
                                             scale=dit[:, bi:bi + 1])
                nc.gpsimd.dma_start(
                    t2_loc[q][:NDST, :].rearrange("(b p) f -> p b f", p=128),
                    big[:])
                if with_collective:
                    nc.gpsimd.collective_compute(
                        "AllGather", ALU.bypass,
                        replica_groups=[list(range(N_CORES))],
                        ins=[t2_loc[q][:NDST, :]], outs=[t2_full[q][:]])

            def l2_chunk(q, t2q):
                """out(q) = sigmoid(W2^T @ (dinv*(M @ t2(q)))^T + b2),
                produced transposed: [(h,c), dst]."""
                for pl in range(PC):
                    ot = pool_ot.tile([128, NDST], bf16, tag="ot")
                    for d0, w in CHAINS:
                        ps = pool_pa.tile([128, FC], f32, tag="pa")
                        for j2 in range(NB2):
                            nc.tensor.matmul(
                                ps[:, :w],
                                a_slice(t2q, j2, pl * 128, 128),
                                mt[j2][:, :, d0:d0 + w],
                                start=(j2 == 0), stop=(j2 == NB2 - 1),
                                perf_mode=DR)
                        s2 = pool_s2.tile([128, FC], bf16, tag="s2")
                        nc.vector.scalar_tensor_tensor(
                            s2[:, :w], ps[:, :w], 1.0, drt[:, d0:d0 + w],
                            ALU.mult, ALU.mult)
                        p2 = pool_p2.tile([128, FC], f32, tag="p2")
                        nc.tensor.matmul(p2[:, :w], w2t[:], s2[:, :w],
                                         start=True, stop=True)
                        nc.scalar.activation(ot[:, d0:d0 + w], p2[:, :w],
                                             AF.Sigmoid, bias=b2t[:])
                    nc.gpsimd.dma_start(out_ext[q * PC + pl], ot[:])

            # ---- pipelined emission ----
            # (ordering of quad-set acquisitions is load-bearing: the pool
            #  holds 2 sets; see module docstring)
            full, loc = (xw_full, xw_loc) if with_collective else (None, xw_loc)
            tfull, tloc = (t2_full, t2_loc) if with_collective else (None, t2_loc)
            w1_chunk(0)
            xwq0 = quad_load(0, full, loc)
            w1_chunk(1)
            w1_chunk(2)
            xwq1 = quad_load(1, full, loc)
            l1a_chunk(0, xwq0)
            t2q0 = quad_load(0, tfull, tloc)
            nc.sync.dma_start(drt[:], dr_ext[:])
            l1a_chunk(1, xwq1)
            with tc.tile_wait_until(0.105):
                xwq2 = quad_load(2, full, loc)
            with tc.tile_wait_until(0.150):
                t2q1 = quad_load(1, tfull, tloc)
            l2_chunk(0, t2q0)
            l1a_chunk(2, xwq2)
            t2q2 = quad_load(2, tfull, tloc)
            l2_chunk(1, t2q1)
            l2_chunk(2, t2q2)

    nc.compile()
    return nc


def prepare_inputs(X, edge_index, W1, b1, W2, b2):
    """Host-side graph/layout prep. Returns per-core in_maps."""
    X = np.asarray(X, dtype=np.float32)
    edge_index = np.asarray(edge_index)
    W1 = np.asarray(W1, dtype=np.float32)
    b1 = np.asarray(b1, dtype=np.float32)
    W2 = np.asarray(W2, dtype=np.float32)
    b2 = np.asarray(b2, dtype=np.float32)

    src = edge_index[0].astype(np.int64)
    dst = edge_index[1].astype(np.int64)

    deg = np.bincount(dst, minlength=N).astype(np.float32) + 1.0
    dinv = 1.0 / np.sqrt(deg)
    dinv_pad = np.zeros(NP, np.float32)
    dinv_pad[:N] = dinv

    # M = Adj + I with multiplicity, uint8 counts
    Mfull = np.zeros((NP, NP), np.uint8)
    np.add.at(Mfull, (dst, src), 1)
    Mfull[np.arange(N), np.arange(N)] += 1
    assert Mfull.max() <= 15, "fp8e4 exact-int range exceeded"

    # XB: [NB, 128=(h,cin), PAIRS*128] with dinv-src folded in; s = 2*pl+h
    Xs = X * dinv[None, :, None, None]                  # [B, N, T, C]
    XT = np.zeros((S, C, NP), np.float32)
    XT[:, :, :N] = np.transpose(Xs, (0, 2, 3, 1)).reshape(S, C, N)
    x6 = XT.reshape(PAIRS, 2, C, NB, 128)
    XB = np.ascontiguousarray(np.transpose(x6, (3, 1, 2, 0, 4)))
    XB = XB.reshape(NB, 128, PAIRS * 128).astype(ml_dtypes.bfloat16)

    def blockdiag(W):
        D = np.zeros((128, 128), np.float32)
        D[:64, :64] = W
        D[64:, 64:] = W
        return D.astype(ml_dtypes.bfloat16)

    W1d = blockdiag(W1)
    W2d = blockdiag(W2)
    B1 = np.tile(b1, (128, FC // C)).astype(ml_dtypes.bfloat16)
    B2 = np.concatenate([b2, b2])[:, None].astype(np.float32)

    in_maps = []
    for c in range(N_CORES):
        rows = Mfull[c * NDST:(c + 1) * NDST, :]        # [1280, 10240]
        MT = rows.reshape(NDST, NB2, 2, 128).transpose(1, 3, 2, 0)
        MT = np.ascontiguousarray(MT).reshape(NB2, 128, 2 * NDST)
        MT = MT.astype(ml_dtypes.float8_e4m3)
        sl = dinv_pad[c * NDST:(c + 1) * NDST]
        DI = np.ascontiguousarray(sl.reshape(BPC, 128).T.astype(np.float32))
        DRW = np.ascontiguousarray(np.tile(sl[None, :], (128, 1))
                                   .astype(ml_dtypes.bfloat16))
        in_maps.append({"XB": XB[c * BPC:(c + 1) * BPC], "MT": MT,
                        "W1d": W1d, "W2d": W2d, "B1": B1, "B2": B2,
                        "DI": DI, "DRW": DRW})
    return in_maps


_NC_CACHE = {}


def kernel(X, edge_index, W1, b1, W2, b2):
    if "nc" not in _NC_CACHE:
        _NC_CACHE["nc"] = build_program(with_collective=True)
    nc = _NC_CACHE["nc"]
    in_maps = prepare_inputs(X, edge_index, W1, b1, W2, b2)

    res = None
    for attempt in range(5):
        try:
            res = run_bass_kernel_spmd(nc, in_maps, list(range(N_CORES)))
            break
        except Exception:
            if attempt == 4:
                raise
            time.sleep(60.0 * (attempt + 1))
    assert res is not None

    # reassemble: per core [12, 128, 1280] bf16 -> [24, 64, 1280] f32
    full = np.zeros((S, C, N), np.float32)
    for c in range(N_CORES):
        o = np.asarray(res.results[c]["OUT"]).astype(np.float32)
        o = o.reshape(S, C, NDST)
        lo = c * NDST
        hi = min(N, (c + 1) * NDST)
        if lo < N:
            full[:, :, lo:hi] = o[:, :, :hi - lo]
    out = full.reshape(B, T, C, N).transpose(0, 3, 1, 2)
    return np.ascontiguousarray(out)


# revision 48
# speedup vs baseline: 1.0282x; 1.0282x over previous
"""GCN block (2-layer) Trainium2 Bass kernel.

Math (per B*T slice, shared graph):
  t2 = relu(A @ (X @ W1) + b1);  out = sigmoid(A @ t2 @ W2 + b2)
  A = D^-1/2 (Adj + I) D^-1/2  (PyG gcn_norm, counts edge multiplicity)

Device mapping (per core, 8-way dst-node sharding, N padded 10000->10240):
  * M = Adj + I is applied as dense fp8 (exact small ints) PE matmuls in
    DoubleRow mode (K=256).  Each core owns 10 of the 80 dst node blocks.
    M^T slabs [128 src, 2, 1280 dst] stay SBUF-resident and serve BOTH
    layers: layer 1 uses a [128,2,128] dst-column slice as the stationary
    operand (out = t2, node-major); layer 2 uses the full slab as the
    MOVING operand with the t2 block as stationary, so the A-output lands
    TRANSPOSED ([cols, dst]) and feeds W2 + sigmoid directly — no DRAM
    round-trip / DMA transpose for the W2 stage.
  * W1 is sharded: each core transforms only its 10 src blocks, then the
    fp8 xw activations are AllGather'd; same for the relu'd t2 between
    the layers.  Work is split into 3 column chunks (512 of 1536 cols =
    4 slice-pairs) so the two AllGathers pipeline under the A-stage
    matmuls of neighbouring chunks.  The 6-buffer quad pool doubles as a
    scheduling throttle: later quad-set loads block on tile reuse and
    land just in time, keeping early DMA bandwidth for the M^T stream.
  * dinv factors: src side folded into X on host; dst side applied at the
    layer-1 drain (per-partition scale) and at the layer-2 drain (row-
    replicated dinv tile, elementwise on the free dim).

Timing build (with_collective=False) replaces each AllGather with local
DMA traffic equivalent to what the real path costs the local DMA engines:
the post-collective SBUF loads of the full gathered activations.
"""
import time

import numpy as np
import ml_dtypes

import concourse.bacc as bacc
import concourse.mybir as mybir
import concourse.tile as tile
from concourse.bass_utils import run_bass_kernel_spmd

N_CORES = 8
N = 10000
NP = 10240            # padded nodes
NB = NP // 128        # 80 node blocks
NB2 = NB // 2         # 40 src-block pairs (DoubleRow K=256)
BPC = NB // N_CORES   # 10 dst blocks per core
B, T, C = 2, 12, 64
S = B * T             # 24 slices
F = S * C             # 1536 free columns, col = pl*128 + h*64 + c
PAIRS = S // 2        # 12 slice pairs (s = 2*pl + h)
NCH = 3               # column chunks
FC = F // NCH         # 512 cols = 4 pairs per chunk
PC = PAIRS // NCH     # 4 pairs per chunk
NDST = BPC * 128      # 1280 dst nodes per core
CHAINS = ((0, 512), (512, 512), (1024, 256))  # dst chains for layer 2
QT = 10               # quad tiles per set ([128, 8, FC] each)
QB = NB // QT         # 16 src blocks per quad tile

f32 = mybir.dt.float32
bf16 = mybir.dt.bfloat16
fp8 = mybir.dt.float8e4
DR = mybir.MatmulPerfMode.DoubleRow
AF = mybir.ActivationFunctionType
ALU = mybir.AluOpType


def build_program(with_collective=True, nc_hook=None):
    nc = bacc.Bacc("TRN2", target_bir_lowering=False, debug=False,
                   num_devices=N_CORES)
    if nc_hook is not None:
        nc_hook(nc)

    # X blocks for this core's 10 src blocks: [b][128=(h,cin)][pl*128+node]
    xb_ext = nc.dram_tensor("XB", [BPC, 128, PAIRS * 128], bf16,
                            kind="ExternalInput")
    # M^T slabs: [j2][p_src][e*1280 + dst], fp8 exact ints
    mt_ext = nc.dram_tensor("MT", [NB2, 128, 2 * NDST], fp8,
                            kind="ExternalInput")
    w1_ext = nc.dram_tensor("W1d", [128, 128], bf16, kind="ExternalInput")
    w2_ext = nc.dram_tensor("W2d", [128, 128], bf16, kind="ExternalInput")
    b1_ext = nc.dram_tensor("B1", [128, FC], bf16, kind="ExternalInput")
    b2_ext = nc.dram_tensor("B2", [128, 1], f32, kind="ExternalInput")
    di_ext = nc.dram_tensor("DI", [128, BPC], f32, kind="ExternalInput")
    dr_ext = nc.dram_tensor("DRW", [128, NDST], bf16, kind="ExternalInput")
    out_ext = nc.dram_tensor("OUT", [PAIRS, 128, NDST], bf16,
                             kind="ExternalOutput")

    with tile.TileContext(nc) as tc:
        with (
            tc.tile_pool(name="consts", bufs=1) as consts,
            tc.tile_pool(name="mt", bufs=NB2) as pool_mt,
            tc.tile_pool(name="xb", bufs=4) as pool_xb,
            tc.tile_pool(name="quads", bufs=9) as pool_q,
            tc.tile_pool(name="stage", bufs=2) as pool_st,
            tc.tile_pool(name="u", bufs=2) as pool_u,
            tc.tile_pool(name="s2", bufs=3) as pool_s2,
            tc.tile_pool(name="ot", bufs=3) as pool_ot,
            tc.tile_pool(name="pa", bufs=6, space="PSUM") as pool_pa,
            tc.tile_pool(name="p2", bufs=2, space="PSUM") as pool_p2,
            tc.tile_pool(name="dram", bufs=1, space="DRAM") as dram,
        ):
            # ---- constants ----
            w1t = consts.tile([128, 128], bf16, tag="w1")
            nc.sync.dma_start(w1t[:], w1_ext[:])
            w2t = consts.tile([128, 128], bf16, tag="w2")
            nc.sync.dma_start(w2t[:], w2_ext[:])
            b1t = consts.tile([128, FC], bf16, tag="b1")
            nc.sync.dma_start(b1t[:], b1_ext[:])
            b2t = consts.tile([128, 1], f32, tag="b2")
            nc.sync.dma_start(b2t[:], b2_ext[:])
            dit = consts.tile([128, BPC], f32, tag="di")
            nc.sync.dma_start(dit[:], di_ext[:])
            # ---- M^T slabs, SBUF-resident, serve both layers ----
            mt = []
            for j2 in range(NB2):
                m = pool_mt.tile([128, 2, NDST], fp8, tag="mt")
                nc.sync.dma_start(m[:].rearrange("p a d -> p (a d)"),
                                  mt_ext[j2])
                mt.append(m)
            drt = consts.tile([128, NDST], bf16, tag="dr")
            with tc.tile_wait_until(0.100):
                nc.sync.dma_start(drt[:], dr_ext[:])

            # ---- DRAM intermediates (per chunk) ----
            # over-allocated to QB*128 rows so the timing build's AllGather
            # stand-in can source a full quad tile in one DMA
            LR = max(QB * 128, NDST)
            xw_loc = [dram.tile([LR, FC], fp8, tag=f"xwl{q}",
                                name=f"xwl{q}") for q in range(NCH)]
            t2_loc = [dram.tile([LR, FC], fp8, tag=f"t2l{q}",
                                name=f"t2l{q}") for q in range(NCH)]
            if with_collective:
                xw_full = [dram.tile([NP, FC], fp8, tag=f"xwf{q}", name=f"xwf{q}",
                                     addr_space="Shared")
                           for q in range(NCH)]
                t2_full = [dram.tile([NP, FC], fp8, tag=f"t2f{q}", name=f"t2f{q}",
                                     addr_space="Shared")
                           for q in range(NCH)]

            def w1_chunk(q, xb_wait=None):
                """xw(q) = (X @ W1) for this core's 10 blocks, cols of q."""
                big = pool_st.tile([128, BPC, FC], fp8, tag="st")
                for h in range(2):
                    xb = pool_xb.tile([128, BPC // 2, FC], bf16, tag="xb")
                    with tc.tile_wait_until(xb_wait or 0,
                                            enable=xb_wait is not None):
                        nc.scalar.dma_start(
                            xb[:],
                            xb_ext[h * 5:h * 5 + 5, :, q * FC:(q + 1) * FC]
                            .rearrange("a p d -> p a d"))
                    for i in range(BPC // 2):
                        b = h * 5 + i
                        ps = pool_p2.tile([128, FC], f32, tag="p2")
                        for pl in range(PC):
                            nc.tensor.matmul(
                                ps[:, pl * 128:(pl + 1) * 128],
                                xb[:, i, pl * 128:(pl + 1) * 128], w1t[:],
                                start=True, stop=True)
                        if b % 2 == 0:
                            nc.vector.tensor_scalar_mul(big[:, b, :], ps[:],
                                                        1.0)
                        else:
                            nc.scalar.activation(big[:, b, :], ps[:], AF.Copy)
                    nc.scalar.dma_start(
                        xw_loc[q][h * 640:(h + 1) * 640, :]
                        .rearrange("(b p) f -> p b f", p=128),
                        big[:, h * 5:(h + 1) * 5, :])
                if with_collective:
                    nc.gpsimd.collective_compute(
                        "AllGather", ALU.bypass,
                        replica_groups=[list(range(N_CORES))],
                        ins=[xw_loc[q][:NDST, :]], outs=[xw_full[q][:]])

            def quad_load(q, full, loc):
                """Load the gathered [NP, FC] activations into a 5-tile quad
                set.  Timing build: equivalent local-DMA traffic sourced from
                the local shard (content unused for timing)."""
                tiles = []
                for g in range(QT):
                    qt = pool_q.tile([128, QB, FC], fp8, tag="quad")
                    eng = nc.scalar if g % 2 == 0 else nc.sync
                    if with_collective:
                        eng.dma_start(
                            qt[:],
                            full[q][g * QB * 128:(g + 1) * QB * 128, :]
                            .rearrange("(a p) f -> p a f", p=128))
                    else:
                        eng.dma_start(
                            qt[:],
                            loc[q][:QB * 128, :]
                            .rearrange("(a p) f -> p a f", p=128))
                    tiles.append(qt)
                return tiles

            def a_slice(tiles, j2, c0, w):
                g, a = (2 * j2) // QB, (2 * j2) % QB
                return tiles[g][:, a:a + 2, c0:c0 + w]

            def l1a_chunk(q, xwq, mid=None):
                """t2(q) = dinv * relu(dinv * (M @ xw(q)) + b1), node-major."""
                big = pool_st.tile([128, BPC, FC], fp8, tag="st")
                for g0 in (0, 5):
                    if g0 == 5 and mid is not None:
                        mid()
                    pss = [pool_pa.tile([128, FC], f32, tag="pa", name=f"pa{q}_{g0}_{i}")
                           for i in range(5)]
                    for j2 in range(NB2):
                        for i in range(5):
                            bi = g0 + i
                            nc.tensor.matmul(
                                pss[i][:],
                                mt[j2][:, :, bi * 128:(bi + 1) * 128],
                                a_slice(xwq, j2, 0, FC),
                                start=(j2 == 0), stop=(j2 == NB2 - 1),
                                perf_mode=DR)
                    for i in range(5):
                        bi = g0 + i
                        u = pool_u.tile([128, FC], bf16, tag="u")
                        nc.vector.scalar_tensor_tensor(
                            u[:], pss[i][:], dit[:, bi:bi + 1], b1t[:],
                            ALU.mult, ALU.add)
                        nc.scalar.activation(big[:, bi, :], u[:], AF.Relu,
                                             scale=dit[:, bi:bi + 1])
                nc.gpsimd.dma_start(
                    t2_loc[q][:NDST, :].rearrange("(b p) f -> p b f", p=128),
                    big[:])
                if with_collective:
                    nc.gpsimd.collective_compute(
                        "AllGather", ALU.bypass,
                        replica_groups=[list(range(N_CORES))],
                        ins=[t2_loc[q][:NDST, :]], outs=[t2_full[q][:]])

            def l2_chunk(q, t2q):
                """out(q) = sigmoid(W2^T @ (dinv*(M @ t2(q)))^T + b2),
                produced transposed: [(h,c), dst]."""
                for pl in range(PC):
                    ot = pool_ot.tile([128, NDST], bf16, tag="ot")
                    for d0, w in CHAINS:
                        ps = pool_pa.tile([128, FC], f32, tag="pa")
                        for j2 in range(NB2):
                            nc.tensor.matmul(
                                ps[:, :w],
                                a_slice(t2q, j2, pl * 128, 128),
                                mt[j2][:, :, d0:d0 + w],
                                start=(j2 == 0), stop=(j2 == NB2 - 1),
                                perf_mode=DR)
                        s2 = pool_s2.tile([128, FC], bf16, tag="s2")
                        nc.vector.scalar_tensor_tensor(
                            s2[:, :w], ps[:, :w], 1.0, drt[:, d0:d0 + w],
                            ALU.mult, ALU.mult)
                        p2 = pool_p2.tile([128, FC], f32, tag="p2")
                        nc.tensor.matmul(p2[:, :w], w2t[:], s2[:, :w],
                                         start=True, stop=True)
                        nc.scalar.activation(ot[:, d0:d0 + w], p2[:, :w],
                                             AF.Sigmoid, bias=b2t[:])
                    nc.gpsimd.dma_start(out_ext[q * PC + pl], ot[:])

            # ---- pipelined emission ----
            # (ordering of quad-set acquisitions is load-bearing: the pool
            #  holds 2 sets; see module docstring)
            full, loc = (xw_full, xw_loc) if with_collective else (None, xw_loc)
            tfull, tloc = (t2_full, t2_loc) if with_collective else (None, t2_loc)
            w1_chunk(0)
            xwq0 = quad_load(0, full, loc)
            w1_chunk(1)
            w1_chunk(2)
            xwq1 = quad_load(1, full, loc)
            l1a_chunk(0, xwq0)
            t2q0 = quad_load(0, tfull, tloc)
            nc.sync.dma_start(drt[:], dr_ext[:])
            l1a_chunk(1, xwq1)
            with tc.tile_wait_until(0.105):
                xwq2 = quad_load(2, full, loc)
            with tc.tile_wait_until(0.150):
                t2q1 = quad_load(1, tfull, tloc)
            l2_chunk(0, t2q0)
            l1a_chunk(2, xwq2)
            t2q2 = quad_load(2, tfull, tloc)
            l2_chunk(1, t2q1)
            l2_chunk(2, t2q2)

    nc.compile()
    return nc


def prepare_inputs(X, edge_index, W1, b1, W2, b2):
    """Host-side graph/layout prep. Returns per-core in_maps."""
    X = np.asarray(X, dtype=np.float32)
    edge_index = np.asarray(edge_index)
    W1 = np.asarray(W1, dtype=np.float32)
    b1 = np.asarray(b1, dtype=np.float32)
    W2 = np.asarray(W2, dtype=np.float32)
    b2 = np.asarray(b2, dtype=np.float32)

    src = edge_index[0].astype(np.int64)
    dst = edge_index[1].astype(np.int64)

    deg = np.bincount(dst, minlength=N).astype(np.float32) + 1.0
    dinv = 1.0 / np.sqrt(deg)
    dinv_pad = np.zeros(NP, np.float32)
    dinv_pad[:N] = dinv

    # M = Adj + I with multiplicity, uint8 counts
    Mfull = np.zeros((NP, NP), np.uint8)
    np.add.at(Mfull, (dst, src), 1)
    Mfull[np.arange(N), np.arange(N)] += 1
    assert Mfull.max() <= 15, "fp8e4 exact-int range exceeded"

    # XB: [NB, 128=(h,cin), PAIRS*128] with dinv-src folded in; s = 2*pl+h
    Xs = X * dinv[None, :, None, None]                  # [B, N, T, C]
    XT = np.zeros((S, C, NP), np.float32)
    XT[:, :, :N] = np.transpose(Xs, (0, 2, 3, 1)).reshape(S, C, N)
    x6 = XT.reshape(PAIRS, 2, C, NB, 128)
    XB = np.ascontiguousarray(np.transpose(x6, (3, 1, 2, 0, 4)))
    XB = XB.reshape(NB, 128, PAIRS * 128).astype(ml_dtypes.bfloat16)

    def blockdiag(W):
        D = np.zeros((128, 128), np.float32)
        D[:64, :64] = W
        D[64:, 64:] = W
        return D.astype(ml_dtypes.bfloat16)

    W1d = blockdiag(W1)
    W2d = blockdiag(W2)
    B1 = np.tile(b1, (128, FC // C)).astype(ml_dtypes.bfloat16)
    B2 = np.concatenate([b2, b2])[:, None].astype(np.float32)

    in_maps = []
    for c in range(N_CORES):
        rows = Mfull[c * NDST:(c + 1) * NDST, :]        # [1280, 10240]
        MT = rows.reshape(NDST, NB2, 2, 128).transpose(1, 3, 2, 0)
        MT = np.ascontiguousarray(MT).reshape(NB2, 128, 2 * NDST)
        MT = MT.astype(ml_dtypes.float8_e4m3)
        sl = dinv_pad[c * NDST:(c + 1) * NDST]
        DI = np.ascontiguousarray(sl.reshape(BPC, 128).T.astype(np.float32))
        DRW = np.ascontiguousarray(np.tile(sl[None, :], (128, 1))
                                   .astype(ml_dtypes.bfloat16))
        in_maps.append({"XB": XB[c * BPC:(c + 1) * BPC], "MT": MT,
                        "W1d": W1d, "W2d": W2d, "B1": B1, "B2": B2,
                        "DI": DI, "DRW": DRW})
    return in_maps


_NC_CACHE = {}


def kernel(X, edge_index, W1, b1, W2, b2):
    if "nc" not in _NC_CACHE:
        _NC_CACHE["nc"] = build_program(with_collective=True)
    nc = _NC_CACHE["nc"]
    in_maps = prepare_inputs(X, edge_index, W1, b1, W2, b2)

    res = None
    for attempt in range(5):
        try:
            res = run_bass_kernel_spmd(nc, in_maps, list(range(N_CORES)))
            break
        except Exception:
            if attempt == 4:
                raise
            time.sleep(60.0 * (attempt + 1))
    assert res is not None

    # reassemble: per core [12, 128, 1280] bf16 -> [24, 64, 1280] f32
    full = np.zeros((S, C, N), np.float32)
    for c in range(N_CORES):
        o = np.asarray(res.results[c]["OUT"]).astype(np.float32)
        o = o.reshape(S, C, NDST)
        lo = c * NDST
        hi = min(N, (c + 1) * NDST)
        if lo < N:
            full[:, :, lo:hi] = o[:, :, :hi - lo]
    out = full.reshape(B, T, C, N).transpose(0, 3, 1, 2)
    return np.ascontiguousarray(out)
